# revision 17
# baseline (speedup 1.0000x reference)
"""KGE (TransR-style) loss kernel for Trainium2, 8 NeuronCores.

Strategy (v2):
  - Host: choose a relation ORDER such that after sorting rows by relation,
    no 128-row tile contains more than one relation boundary (greedy over
    prefix sums). Rows split 1024/core -> 8 tiles/core, ZERO padding.
    Entity table converted to bf16 host-side (halves gather DMA traffic).
  - Mixed-relation tiles are handled with data (not structure): per tile j
    the program computes  vT = W_A^T dT + W_D^T (dT*mask) + [r_A; r_D]-add
    where W_D = W_B - W_A and mask is the suffix indicator of the boundary
    row. Tiles without a boundary get W_D = 0. One fixed SPMD program.
  - Device (per core):
      * 2 batched indirect DMAs gather h/pos/neg rows (2-tile group then
        6-tile group so compute starts early).  Replaces the original 36
        gathers whose ~1us/SWDGE fixed cost dominated the baseline.
      * DVE/Pool: D_p = H-P, D_n = H-N (bf16, 2x mode)
      * PE: transpose D tiles (bf16 pairs in one PSUM group)
      * ACT+DVE: PSUM->SBUF pair copies;  DVE: masked copies (dT*mask)
      * PE: per tile, 3-matmul PSUM group -> [v_pos | v_neg] (f32 PSUM)
      * ACT: Square -> sq (f32)
      * PE: score diffs via  transpose(sq_neg) - sq_pos^T  (transpose +
        negated-identity matmul in one PSUM group, exact f32)
      * DVE: reduce -> per-row score diffs [128, 8]
      * softplus tail on [128,8] (Abs/Exp/Ln on ACT, relu-mul on DVE)
      * reg terms via Square/ttr accumulators split across ACT/DVE
      * final: reduce + ones-matmul -> one f32 per core; host sums / M.
  - PE warmup transposes run during the gather latency (p-state ramp), and
    a dummy Ln/Exp pair pins the activation table early so no table load
    lands in the critical tail.
"""

import os
from contextlib import ExitStack

import numpy as np

import concourse.bass as bass
import concourse.tile as tile
from concourse import bacc, mybir
from concourse.masks import make_identity

M = 8192
E = 128
N_ENT = 500000
N_REL = 64
LAM = 1e-5
P = 128
N_CORES = 8
RPC = M // N_CORES          # rows per core = 1024
NT = RPC // P               # tiles per core = 8
GROUPS = [(0, 2), (2, 8)]   # gather groups (tile ranges)
N_WARM = 34                 # PE warmup transposes

f32 = mybir.dt.float32
bf16 = mybir.dt.bfloat16
i32 = mybir.dt.int32

_cache = {}

# ---- i32 blob column layout ----
_B_IDX = 0                    # [128, 3*NT] i32 gather indices (grouped)
_B_MCOL = _B_IDX + 3 * NT     # [128, NT] f32 suffix masks
_B_CNT = _B_MCOL + NT         # [128, 1] f32 cnt (rows 0:64 used)
_B_RFULL = _B_CNT + 1         # [128, E] f32 relation_embed (rows 0:64)
_B_COLS = _B_RFULL + E

# ---- bf16 rsm blob layout: [2, 3*NT*128] ----
_R_RR = 0                     # row0 = r_A, row1 = r_D per tile
_R_RRHS = NT * P              # row0 = ones, row1 = [mrow|mrow] per tile


def _build():
    nc = bacc.Bacc(
        "TRN2",
        target_bir_lowering=False,
        debug=False,
        num_devices=N_CORES,
    )

    ent = nc.dram_tensor("ent", (N_ENT, E), bf16, kind="ExternalInput").ap()
    blob = nc.dram_tensor("blob", (P, _B_COLS), i32, kind="ExternalInput").ap()
    wab = nc.dram_tensor("wab", (P, 2 * NT * P), bf16, kind="ExternalInput").ap()
    rsm = nc.dram_tensor("rsm", (2, 3 * NT * P), bf16, kind="ExternalInput").ap()
    mful = nc.dram_tensor("mful", (P, 2 * NT * P), bf16, kind="ExternalInput").ap()
    out = nc.dram_tensor("out", (1, 1), f32, kind="ExternalOutput").ap()
    debug = bool(os.environ.get("KGE_DEBUG"))
    if debug:
        dbg_dsc = nc.dram_tensor("dbg_dsc", (P, NT), f32, kind="ExternalOutput").ap()
        dbg_acc = nc.dram_tensor("dbg_acc", (P, 16), f32, kind="ExternalOutput").ap()

    with tile.TileContext(nc) as tc, ExitStack() as ctx:
        const = ctx.enter_context(tc.tile_pool(name="const", bufs=1))
        ps_pair = ctx.enter_context(tc.tile_pool(name="ps_pair", bufs=3, space="PSUM"))
        ps_vt = ctx.enter_context(tc.tile_pool(name="ps_vt", bufs=2, space="PSUM"))
        ps_sc = ctx.enter_context(tc.tile_pool(name="ps_sc", bufs=2, space="PSUM"))
        ps_fin = ctx.enter_context(tc.tile_pool(name="ps_fin", bufs=1, space="PSUM"))

        # ---- small inputs ----
        blob_sb = const.tile([P, _B_COLS], i32)
        nc.sync.dma_start(out=blob_sb[:], in_=blob[:])
        wab_sb = const.tile([P, 2 * NT * P], bf16)
        nc.sync.dma_start(out=wab_sb[:], in_=wab[:])
        rsm_sb = const.tile([2, 3 * NT * P], bf16)
        nc.sync.dma_start(out=rsm_sb[:], in_=rsm[:])
        mf_sb = const.tile([P, 2 * NT * P], bf16)
        nc.sync.dma_start(out=mf_sb[:], in_=mful[:])
        mf3 = mf_sb[:].rearrange("p (t c) -> p t c", t=2)

        idx_sb = blob_sb[:, _B_IDX : _B_IDX + 3 * NT]
        mcol_sb = blob_sb[:, _B_MCOL : _B_MCOL + NT].bitcast(f32)
        cnt_sb = blob_sb[:, _B_CNT : _B_CNT + 1].bitcast(f32)
        rfull_sb = blob_sb[:, _B_RFULL : _B_RFULL + E].bitcast(f32)

        # ---- constants ----
        iden = const.tile([P, P], f32)
        make_identity(nc, iden[:])
        iden_bf = const.tile([P, P], bf16)
        nc.scalar.copy(iden_bf[:], iden[:])
        negiden = const.tile([P, P], f32)
        nc.vector.tensor_scalar_mul(out=negiden[:], in0=iden[:], scalar1=-1.0)
        ones_col = const.tile([P, 1], f32)
        nc.gpsimd.memset(ones_col[:], 1.0)
        zeros8 = const.tile([P, NT], f32)
        nc.gpsimd.memset(zeros8[:], 0.0)
        acc = const.tile([P, 16], f32)
        nc.vector.memset(acc[:], 0.0)

        # pin the activation table that has Copy/Square/Abs/Exp/Ln/Relu
        # (natural_log_exp_and_others) before the compute phase
        dummy = const.tile([1, 2], f32)
        nc.scalar.activation(
            out=dummy[:, 0:1], in_=ones_col[0:1, :],
            func=mybir.ActivationFunctionType.Ln,
        )
        nc.scalar.activation(
            out=dummy[:, 1:2], in_=ones_col[0:1, :],
            func=mybir.ActivationFunctionType.Exp, scale=-1.0,
        )

        # ---- PE warmup (p-state ramp) ----
        wu_src = const.tile([P, P], bf16)
        nc.gpsimd.memset(wu_src[:], 0.0)
        for _ in range(N_WARM):
            wu_ps = ps_pair.tile([P, P], bf16, tag="pair")
            nc.tensor.matmul(
                out=wu_ps[:], lhsT=wu_src[:], rhs=iden_bf[:],
                is_transpose=True, start=True, stop=True,
            )

        # ---- gathers: X = [H | P | N] (each [128, RPC] bf16) ----
        X = const.tile([P, 3 * RPC], bf16)
        X3 = X[:].rearrange("p (r c) -> p r c", r=3)
        for (t0, t1) in GROUPS:
            nc.gpsimd.indirect_dma_start(
                out=X3[:, :, t0 * P : t1 * P],
                out_offset=None,
                in_=ent[:],
                in_offset=bass.IndirectOffsetOnAxis(
                    ap=idx_sb[:, 3 * t0 : 3 * t1], axis=0
                ),
            )

        xh = X[:, 0:RPC]
        xp = X[:, RPC : 2 * RPC]
        xn = X[:, 2 * RPC : 3 * RPC]

        # ---- working tiles ----
        dp = const.tile([P, RPC], bf16)
        dn = const.tile([P, RPC], bf16)
        dT = const.tile([P, 2, RPC], bf16)    # [pos | neg] transposed
        dTm = const.tile([P, 2, RPC], bf16)   # masked
        sq = const.tile([P, 2, RPC], f32)
        dsc = const.tile([P, NT], f32)
        xsq_a = const.tile([P, 3 * RPC // 2], f32)   # ACT square scratch
        xsq_b = const.tile([P, 3 * RPC // 2], f32)   # DVE ttr scratch

        for gi, (t0, t1) in enumerate(GROUPS):
            cs = slice(t0 * P, t1 * P)
            gslab = X3[:, :, cs]
            ncols = 3 * (t1 - t0) * P

            # reg partials (col 8+gi*2 / 9+gi*2 of acc)
            if gi == 0:
                nc.vector.tensor_tensor_reduce(
                    out=xsq_b[:, 0:ncols], in0=gslab, in1=gslab,
                    scale=1.0, scalar=0.0,
                    op0=mybir.AluOpType.mult, op1=mybir.AluOpType.add,
                    accum_out=acc[:, 8:9],
                )
            else:
                half = (t1 - t0) * P // 2
                nc.scalar.activation(
                    out=xsq_a[:, 0 : 3 * half], in_=X3[:, :, t0 * P : t0 * P + half],
                    func=mybir.ActivationFunctionType.Square,
                    accum_out=acc[:, 9:10],
                )
                nc.vector.tensor_tensor_reduce(
                    out=xsq_b[:, 0 : 3 * half],
                    in0=X3[:, :, t0 * P + half : t1 * P],
                    in1=X3[:, :, t0 * P + half : t1 * P],
                    scale=1.0, scalar=0.0,
                    op0=mybir.AluOpType.mult, op1=mybir.AluOpType.add,
                    accum_out=acc[:, 10:11],
                )

            # D_p / D_n (bf16 in -> 2x mode), g1 split DVE/Pool
            if gi == 0:
                nc.vector.tensor_tensor(
                    out=dp[:, cs], in0=xh[:, cs], in1=xp[:, cs],
                    op=mybir.AluOpType.subtract,
                )
                nc.vector.tensor_tensor(
                    out=dn[:, cs], in0=xh[:, cs], in1=xn[:, cs],
                    op=mybir.AluOpType.subtract,
                )
            else:
                nc.vector.tensor_tensor(
                    out=dp[:, cs], in0=xh[:, cs], in1=xp[:, cs],
                    op=mybir.AluOpType.subtract,
                )
                nc.gpsimd.tensor_tensor(
                    out=dn[:, cs], in0=xh[:, cs], in1=xn[:, cs],
                    op=mybir.AluOpType.subtract,
                )

            for j in range(t0, t1):
                js = slice(j * P, (j + 1) * P)
                # transpose pair -> one PSUM group [128, 2, 128] bf16
                pair = ps_pair.tile([P, 2, P], bf16, tag="pair")
                nc.tensor.matmul(
                    out=pair[:, 0, :], lhsT=dp[:, js], rhs=iden_bf[:],
                    is_transpose=True, start=True, stop=False,
                )
                nc.tensor.matmul(
                    out=pair[:, 1, :], lhsT=dn[:, js], rhs=iden_bf[:],
                    is_transpose=True, start=False, stop=True,
                )
                # PSUM -> SBUF copy (ACT/DVE split) and masked copy (DVE)
                if j in (0, 3, 6):
                    nc.scalar.copy(dT[:, :, js], pair[:])
                else:
                    nc.vector.tensor_copy(dT[:, :, js], pair[:])
                nc.vector.tensor_tensor(
                    out=dTm[:, :, js], in0=dT[:, :, js], in1=mf3[:, :, js],
                    op=mybir.AluOpType.mult,
                )

                # vT = W_A^T dT + W_D^T dTm + [r_A; r_D] x [ones; mrow]
                vt = ps_vt.tile([P, 2 * P], f32, tag="vt")
                nc.tensor.matmul(
                    out=vt[:], lhsT=wab_sb[:, js], rhs=dT[:, :, js],
                    start=True, stop=False,
                )
                nc.tensor.matmul(
                    out=vt[:],
                    lhsT=wab_sb[:, (NT + j) * P : (NT + j + 1) * P],
                    rhs=dTm[:, :, js],
                    start=False, stop=False,
                )
                nc.tensor.matmul(
                    out=vt[:],
                    lhsT=rsm_sb[0:2, _R_RR + j * P : _R_RR + (j + 1) * P],
                    rhs=rsm_sb[0:2, _R_RRHS + j * 2 * P : _R_RRHS + (j + 1) * 2 * P],
                    start=False, stop=True,
                )

                # sq[:, :, js] = vt^2   (f32: avoids cancellation error)
                nc.scalar.activation(
                    out=sq[:, :, js], in_=vt[:],
                    func=mybir.ActivationFunctionType.Square,
                )

                # score-diff transposes: every 2 tiles, one PSUM group:
                #   scp[:, k, :] = sq_neg_j^T - sq_pos_j^T   (exact f32)
                if j % 2 == 1:
                    scp = ps_sc.tile([P, 2, P], f32, tag="scp")
                    for k, jj in enumerate((j - 1, j)):
                        jjs = slice(jj * P, (jj + 1) * P)
                        nc.tensor.matmul(
                            out=scp[:, k, :], lhsT=sq[:, 1, jjs], rhs=iden[:],
                            is_transpose=True,
                            start=(k == 0), stop=False,
                        )
                        nc.tensor.matmul(
                            out=scp[:, k, :], lhsT=sq[:, 0, jjs], rhs=negiden[:],
                            start=False, stop=(k == 1),
                        )
                    nc.vector.tensor_reduce(
                        out=dsc[:, j - 1 : j + 1], in_=scp[:],
                        axis=mybir.AxisListType.X, op=mybir.AluOpType.add,
                    )

        # ---- softplus tail: softplus(0.5*dsc) -> acc[:, 0:8] ----
        t_abs = const.tile([P, NT], f32)
        nc.scalar.activation(
            out=t_abs[:], in_=dsc[:], func=mybir.ActivationFunctionType.Abs,
            scale=0.5,
        )
        t_exp = const.tile([P, NT], f32)
        nc.scalar.activation(
            out=t_exp[:], in_=t_abs[:], func=mybir.ActivationFunctionType.Exp,
            scale=-1.0,
        )
        t_ln = const.tile([P, NT], f32)
        nc.scalar.activation(
            out=t_ln[:], in_=t_exp[:], func=mybir.ActivationFunctionType.Ln,
            bias=1.0,
        )
        t_relu = const.tile([P, NT], f32)
        nc.vector.scalar_tensor_tensor(
            out=t_relu[:], in0=dsc[:], scalar=0.5, in1=zeros8[:],
            op0=mybir.AluOpType.mult, op1=mybir.AluOpType.max,
        )
        nc.vector.tensor_tensor(
            out=acc[:, 0:NT], in0=t_ln[:], in1=t_relu[:], op=mybir.AluOpType.add
        )

        # ---- reg terms ----
        nc.vector.tensor_scalar_mul(
            out=acc[:, 8:11], in0=acc[:, 8:11], scalar1=0.5 * LAM
        )
        rsq_scr = const.tile([64, E], f32)
        rsq_col = const.tile([64, 1], f32)
        nc.vector.tensor_tensor_reduce(
            out=rsq_scr[:], in0=rfull_sb[0:64, :], in1=rfull_sb[0:64, :],
            scale=1.0, scalar=0.0,
            op0=mybir.AluOpType.mult, op1=mybir.AluOpType.add,
            accum_out=rsq_col[:],
        )
        nc.vector.tensor_tensor(
            out=acc[0:64, 12:13], in0=rsq_col[:], in1=cnt_sb[0:64, :],
            op=mybir.AluOpType.mult,
        )

        # ---- final scalar ----
        t_col = const.tile([P, 1], f32)
        nc.vector.tensor_reduce(
            out=t_col[:], in_=acc[:], axis=mybir.AxisListType.X,
            op=mybir.AluOpType.add,
        )
        fin_ps = ps_fin.tile([1, 1], f32)
        nc.tensor.matmul(
            out=fin_ps[:], lhsT=t_col[:], rhs=ones_col[:], start=True, stop=True
        )
        fin_sb = const.tile([1, 1], f32)
        nc.scalar.copy(fin_sb[:], fin_ps[:])
        nc.sync.dma_start(out=out[:], in_=fin_sb[:])
        if debug:
            nc.sync.dma_start(out=dbg_dsc[:], in_=dsc[:])
            nc.sync.dma_start(out=dbg_acc[:], in_=acc[:])

    nc.compile()
    return nc


def _choose_order(counts):
    """Permute relations so no 128-row window holds 2 boundaries."""
    rng = np.random.RandomState(0)
    for attempt in range(200):
        remaining = set(range(N_REL))
        orderp = []
        p = 0
        ok = True
        while remaining:
            cands = []
            for k in remaining:
                c = int(counts[k])
                viol = (
                    p > 0
                    and p + c < M
                    and (p // P) == ((p + c) // P)
                )
                if not viol:
                    cands.append((((p + c) % P), k))
            if not cands:
                ok = False
                break
            if attempt == 0:
                cands.sort()
                k = cands[-1][1]
            else:
                k = cands[rng.randint(len(cands))][1]
            orderp.append(k)
            p += int(counts[k])
            remaining.discard(k)
        if ok:
            return orderp
    raise RuntimeError("could not find a 1-boundary-per-tile relation order")


def _plan(h, r, pos_t, neg_t, relation_weight, relation_embed):
    counts = np.bincount(r, minlength=N_REL)
    perm = _choose_order(counts)
    order = np.concatenate(
        [np.flatnonzero(r == k) for k in perm if counts[k] > 0]
    ).astype(np.int64)
    assert order.shape[0] == M
    h_s = h[order]
    p_s = pos_t[order]
    n_s = neg_t[order]
    r_s = r[order]

    rw = relation_weight.astype(np.float32)
    re = relation_embed.astype(np.float32)

    import ml_dtypes

    maps = []
    for c in range(N_CORES):
        rows = slice(c * RPC, (c + 1) * RPC)
        hh = h_s[rows].reshape(NT, P).T.astype(np.int32)   # [128, NT]
        pp = p_s[rows].reshape(NT, P).T.astype(np.int32)
        nn = n_s[rows].reshape(NT, P).T.astype(np.int32)
        rc = r_s[rows]

        # grouped idx layout: per gather group [H tiles.., P tiles.., N tiles..]
        idx = np.zeros((P, 3 * NT), np.int32)
        for (t0, t1) in GROUPS:
            k = t1 - t0
            base = 3 * t0
            idx[:, base : base + k] = hh[:, t0:t1]
            idx[:, base + k : base + 2 * k] = pp[:, t0:t1]
            idx[:, base + 2 * k : base + 3 * k] = nn[:, t0:t1]

        wab = np.zeros((P, 2 * NT * P), np.float32)
        rr = np.zeros((2, NT * P), np.float32)
        rrhs = np.zeros((2, NT * 2 * P), np.float32)
        mcol = np.zeros((P, NT), np.float32)
        mful = np.zeros((2, NT * P), np.float32)  # [role, global col] mask
        for j in range(NT):
            tr = rc[j * P : (j + 1) * P]
            rel_a = int(tr[0])
            chg = np.flatnonzero(tr[1:] != tr[:-1])
            assert len(chg) <= 1, "tile with >1 relation boundary"
            wab[:, j * P : (j + 1) * P] = rw[rel_a]
            rr[0, j * P : (j + 1) * P] = re[rel_a]
            rrhs[0, j * 2 * P : (j + 1) * 2 * P] = 1.0
            if len(chg) == 1:
                b = int(chg[0]) + 1
                rel_b = int(tr[b])
                wab[:, (NT + j) * P : (NT + j + 1) * P] = rw[rel_b] - rw[rel_a]
                rr[1, j * P : (j + 1) * P] = re[rel_b] - re[rel_a]
                mrow = np.zeros(P, np.float32)
                mrow[b:] = 1.0
                rrhs[1, j * 2 * P : j * 2 * P + P] = mrow
                rrhs[1, j * 2 * P + P : (j + 1) * 2 * P] = mrow
                mful[:, j * P : (j + 1) * P] = mrow[None, :]
                mcol[b:, j] = 1.0

        cnt = np.zeros((P, 1), np.float32)
        core_counts = np.bincount(rc, minlength=N_REL)
        cnt[:N_REL, 0] = core_counts * (0.5 * LAM)
        rfull = np.zeros((P, E), np.float32)
        rfull[:N_REL] = re

        blob = np.zeros((P, _B_COLS), np.int32)
        blob[:, _B_IDX : _B_IDX + 3 * NT] = idx
        blob[:, _B_MCOL : _B_MCOL + NT] = mcol.view(np.int32)
        blob[:, _B_CNT : _B_CNT + 1] = cnt.view(np.int32)
        blob[:, _B_RFULL : _B_RFULL + E] = rfull.view(np.int32)

        rsm = np.zeros((2, 3 * NT * P), np.float32)
        rsm[:, _R_RR : _R_RR + NT * P] = rr
        rsm[:, _R_RRHS : _R_RRHS + 2 * NT * P] = rrhs

        maps.append(
            {
                "blob": blob,
                "wab": wab.astype(ml_dtypes.bfloat16),
                "rsm": rsm.astype(ml_dtypes.bfloat16),
                "mful": np.broadcast_to(
                    mful.reshape(1, 2 * NT * P), (P, 2 * NT * P)
                ).astype(ml_dtypes.bfloat16),
            }
        )
    return maps


def kernel(h, r, pos_t, neg_t, entity_embed, relation_embed, relation_weight):
    import ml_dtypes

    h = np.asarray(h).astype(np.int64)
    r = np.asarray(r).astype(np.int64)
    pos_t = np.asarray(pos_t).astype(np.int64)
    neg_t = np.asarray(neg_t).astype(np.int64)
    ent = np.ascontiguousarray(
        np.asarray(entity_embed, dtype=np.float32).astype(ml_dtypes.bfloat16)
    )
    re = np.ascontiguousarray(np.asarray(relation_embed, dtype=np.float32))
    rw = np.ascontiguousarray(np.asarray(relation_weight, dtype=np.float32))

    maps = _plan(h, r, pos_t, neg_t, rw, re)
    if "nc" not in _cache:
        _cache["nc"] = _build()
    nc = _cache["nc"]

    in_maps = [{"ent": ent, **maps[c]} for c in range(N_CORES)]

    if os.environ.get("KGE_SIM"):
        from concourse.bass_interp import CoreSim

        total = 0.0
        for c in range(N_CORES):
            sim = CoreSim(nc, trace=False)
            for name, arr in in_maps[c].items():
                sim.tensor(name)[:] = arr
            sim.simulate()
            total += float(sim.tensor("out")[0, 0])
        return np.float32(total / M)

    from concourse.bass_utils import run_bass_kernel_spmd

    res = run_bass_kernel_spmd(nc, in_maps, core_ids=list(range(N_CORES)))
    total = sum(float(res.results[c]["out"][0, 0]) for c in range(N_CORES))
    return np.float32(total / M)


# revision 20
# speedup vs baseline: 1.0223x; 1.0223x over previous
"""KGE (TransR-style) loss kernel for Trainium2, 8 NeuronCores.

Strategy (v3):
  - Host: choose a relation ORDER such that after sorting rows by relation,
    no 128-row tile contains more than one relation boundary (greedy over
    prefix sums). Rows split 1024/core -> 8 tiles/core, ZERO padding.
    Entity table converted to bf16 host-side.
  - Mixed-relation tiles are handled with data (not structure): per tile j
    the program computes  vT = W_A^T dT + W_D^T (dT*mask) + [r_A; r_D]-add
    where W_D = W_B - W_A and mask is the suffix indicator of the boundary
    row (a full [128, 2, 1024] bf16 mask tensor, loaded early).
  - Device (per core):
      * 2 batched indirect DMAs gather h/pos/neg rows (2-tile group then
        6-tile group so compute starts early).  Replaces the original 36
        gathers whose ~1us/SWDGE fixed cost dominated the baseline.
      * DVE/Pool: D_p = H-P, D_n = H-N (bf16, 2x mode)
      * PE: transpose D tiles (4 transposes / 2 tiles in one PSUM group)
      * DVE: PSUM->SBUF pair copies + masked copies (dT*mask), 2-tile wide
      * PE: per tile, 3-matmul PSUM group -> [v_pos | v_neg] (f32 PSUM)
      * ACT: Square -> sq (bf16)
      * PE: score diffs via  transpose(sq_neg) - sq_pos^T  (transpose +
        negated-identity matmul in one PSUM group)
      * DVE: reduce -> per-row score diffs [128, 8]
      * ONE Softplus activation for the whole tail (all activation funcs
        used live in the softplus_and_others table -> no mid-kernel
        activation-table loads; a dummy softplus pins the table early)
      * reg terms via ttr/Square accumulators interleaved off-path
      * final: reduce + ones-matmul -> one f32 per core; host sums / M.
  - PE warmup transposes run during the gather latency (p-state ramp).
"""

import os
from contextlib import ExitStack

import numpy as np

import concourse.bass as bass
import concourse.tile as tile
from concourse import bacc, mybir
from concourse.masks import make_identity

# Constrain the activation-table chooser: our funcs (Copy/Square/Abs/Exp/Ln/
# Identity) all exist in natural_log_exp_and_others; removing them from the
# other sets (positions preserved, so act_func_set ids stay valid) makes the
# insert_act_table_loads pass emit exactly one table load, up front, instead
# of one in the critical tail.
_ACT_PATCHED = False


def _patch_act_tables():
    global _ACT_PATCHED
    if _ACT_PATCHED:
        return
    _ACT_PATCHED = True
    orig = bacc.get_activation_tables
    target = "natural_log_exp_and_others"
    ours = {
        mybir.ActivationFunctionType.Copy,
        mybir.ActivationFunctionType.Identity,
        mybir.ActivationFunctionType.Square,
        mybir.ActivationFunctionType.Abs,
        mybir.ActivationFunctionType.Exp,
        mybir.ActivationFunctionType.Ln,
        mybir.ActivationFunctionType.Relu,
    }

    def patched(arch):
        tabs = orig(arch)
        return {
            name: (funcs if name == target else funcs - ours)
            for name, funcs in tabs.items()
        }

    bacc.get_activation_tables = patched

M = 8192
E = 128
N_ENT = 500000
N_REL = 64
LAM = 1e-5
P = 128
N_CORES = 8
RPC = M // N_CORES          # rows per core = 1024
NT = RPC // P               # tiles per core = 8
GROUPS = [(0, 2), (2, 8)]   # gather groups (tile ranges)
N_WARM = 50                 # PE warmup transposes

f32 = mybir.dt.float32
bf16 = mybir.dt.bfloat16
i32 = mybir.dt.int32

_cache = {}

# ---- i32 blob column layout ----
_B_IDX = 0                    # [128, 3*NT] i32 gather indices (grouped)
_B_CNT = _B_IDX + 3 * NT      # [128, 1] f32 cnt (rows 0:64 used)
_B_RFULL = _B_CNT + 1         # [128, E] f32 relation_embed (rows 0:64)
_B_COLS = _B_RFULL + E

# ---- bf16 rsm blob layout: [2, 3*NT*128] ----
_R_RR = 0                     # row0 = r_A, row1 = r_D per tile
_R_RRHS = NT * P              # row0 = ones, row1 = [mrow|mrow] per tile


def _build():
    _patch_act_tables()
    nc = bacc.Bacc(
        "TRN2",
        target_bir_lowering=False,
        debug=False,
        num_devices=N_CORES,
    )

    ent = nc.dram_tensor("ent", (N_ENT, E), bf16, kind="ExternalInput").ap()
    blob = nc.dram_tensor("blob", (P, _B_COLS), i32, kind="ExternalInput").ap()
    wab = nc.dram_tensor("wab", (P, 2 * NT * P), bf16, kind="ExternalInput").ap()
    rsm = nc.dram_tensor("rsm", (2, 3 * NT * P), bf16, kind="ExternalInput").ap()
    mful = nc.dram_tensor("mful", (P, 2 * NT * P), bf16, kind="ExternalInput").ap()
    out = nc.dram_tensor("out", (1, 1), f32, kind="ExternalOutput").ap()
    debug = bool(os.environ.get("KGE_DEBUG"))
    if debug:
        dbg_dsc = nc.dram_tensor("dbg_dsc", (P, NT), f32, kind="ExternalOutput").ap()
        dbg_acc = nc.dram_tensor("dbg_acc", (P, 16), f32, kind="ExternalOutput").ap()

    with tile.TileContext(nc) as tc, ExitStack() as ctx:
        const = ctx.enter_context(tc.tile_pool(name="const", bufs=1))
        ps_pair = ctx.enter_context(tc.tile_pool(name="ps_pair", bufs=3, space="PSUM"))
        ps_vt = ctx.enter_context(tc.tile_pool(name="ps_vt", bufs=2, space="PSUM"))
        ps_sc = ctx.enter_context(tc.tile_pool(name="ps_sc", bufs=2, space="PSUM"))
        ps_fin = ctx.enter_context(tc.tile_pool(name="ps_fin", bufs=1, space="PSUM"))

        # ---- small inputs (blob first: the gathers wait on it) ----
        blob_sb = const.tile([P, _B_COLS], i32)
        nc.sync.dma_start(out=blob_sb[:], in_=blob[:])
        wab_sb = const.tile([P, 2 * NT * P], bf16)
        nc.sync.dma_start(out=wab_sb[:], in_=wab[:])
        rsm_sb = const.tile([2, 3 * NT * P], bf16)
        nc.sync.dma_start(out=rsm_sb[:], in_=rsm[:])
        mf_sb = const.tile([P, 2 * NT * P], bf16)
        # two half-transfers so neither blocks the first gather on the DMA
        # engines for long
        nc.sync.dma_start(out=mf_sb[:, : NT * P], in_=mful[:, : NT * P])
        nc.sync.dma_start(out=mf_sb[:, NT * P :], in_=mful[:, NT * P :])
        mf3 = mf_sb[:].rearrange("p (t c) -> p t c", t=2)

        idx_sb = blob_sb[:, _B_IDX : _B_IDX + 3 * NT]
        cnt_sb = blob_sb[:, _B_CNT : _B_CNT + 1].bitcast(f32)
        rfull_sb = blob_sb[:, _B_RFULL : _B_RFULL + E].bitcast(f32)

        # ---- constants ----
        ones_col = const.tile([P, 1], f32)
        nc.gpsimd.memset(ones_col[:], 1.0)

        zeros8 = const.tile([P, NT], f32)
        nc.gpsimd.memset(zeros8[:], 0.0)

        iden = const.tile([P, P], f32)
        make_identity(nc, iden[:])
        iden_bf = const.tile([P, P], bf16)
        nc.scalar.copy(iden_bf[:], iden[:])
        negiden = const.tile([P, P], f32)
        nc.vector.tensor_scalar_mul(out=negiden[:], in0=iden[:], scalar1=-1.0)
        acc = const.tile([P, 16], f32)
        nc.vector.memset(acc[:], 0.0)

        # ---- PE warmup (p-state ramp) ----
        wu_src = const.tile([P, P], bf16)
        nc.gpsimd.memset(wu_src[:], 0.0)
        for _ in range(N_WARM):
            wu_ps = ps_pair.tile([P, P], bf16, tag="pair")
            nc.tensor.matmul(
                out=wu_ps[:], lhsT=wu_src[:], rhs=iden_bf[:],
                is_transpose=True, start=True, stop=True,
            )

        # ---- gathers: X = [H | P | N] (each [128, RPC] bf16) ----
        X = const.tile([P, 3 * RPC], bf16)
        X3 = X[:].rearrange("p (r c) -> p r c", r=3)
        for (t0, t1) in GROUPS:
            nc.gpsimd.indirect_dma_start(
                out=X3[:, :, t0 * P : t1 * P],
                out_offset=None,
                in_=ent[:],
                in_offset=bass.IndirectOffsetOnAxis(
                    ap=idx_sb[:, 3 * t0 : 3 * t1], axis=0
                ),
            )

        xh = X[:, 0:RPC]
        xp = X[:, RPC : 2 * RPC]
        xn = X[:, 2 * RPC : 3 * RPC]

        # ---- working tiles ----
        dp = const.tile([P, RPC], bf16)
        dn = const.tile([P, RPC], bf16)
        dT = const.tile([P, 2, RPC], bf16)    # [pos | neg] transposed
        dTm = const.tile([P, 2, RPC], bf16)   # masked
        sq = const.tile([P, 2, RPC], f32)
        dsc = const.tile([P, NT], f32)
        xsq_a = const.tile([P, 3 * RPC // 4], f32)   # ACT square scratch
        xsq_b = const.tile([P, 3 * RPC // 4], f32)   # DVE ttr scratch

        xsq_c = const.tile([P, 3 * RPC // 4], f32)   # Pool stt scratch

        def reg_pool(sl, col, scratch_cols):
            gslab = X3[:, :, sl]
            nc.gpsimd.scalar_tensor_tensor(
                out=xsq_c[:, 0:scratch_cols], in0=gslab, scalar=1.0, in1=gslab,
                op0=mybir.AluOpType.mult, op1=mybir.AluOpType.mult,
                accum_out=acc[:, col : col + 1],
            )

        def reg_dve(sl, col, scratch_cols):
            gslab = X3[:, :, sl]
            nc.vector.tensor_tensor_reduce(
                out=xsq_b[:, 0:scratch_cols], in0=gslab, in1=gslab,
                scale=1.0, scalar=0.0,
                op0=mybir.AluOpType.mult, op1=mybir.AluOpType.add,
                accum_out=acc[:, col : col + 1],
            )

        def reg_act(sl, col, scratch_cols):
            nc.scalar.activation(
                out=xsq_a[:, 0:scratch_cols], in_=X3[:, :, sl],
                func=mybir.ActivationFunctionType.Square,
                accum_out=acc[:, col : col + 1],
            )

        for gi, (t0, t1) in enumerate(GROUPS):
            cs = slice(t0 * P, t1 * P)

            # D_p / D_n (bf16 in -> 2x mode); g1's dn goes to Pool
            nc.vector.tensor_tensor(
                out=dp[:, cs], in0=xh[:, cs], in1=xp[:, cs],
                op=mybir.AluOpType.subtract,
            )
            if gi == 0:
                nc.vector.tensor_tensor(
                    out=dn[:, cs], in0=xh[:, cs], in1=xn[:, cs],
                    op=mybir.AluOpType.subtract,
                )
                # g0 reg on DVE: fills the gap while g1 data is in flight
                reg_dve(cs, 8, 3 * 2 * P)
            else:
                nc.gpsimd.tensor_tensor(
                    out=dn[:, cs], in0=xh[:, cs], in1=xn[:, cs],
                    op=mybir.AluOpType.subtract,
                )

            for jj in range(t0, t1, 2):
                # 4 transposes for tiles (jj, jj+1) -> one PSUM group
                # layout [128, 2, 256]: [:, 0] = [dpT_jj | dpT_jj+1]
                pair = ps_pair.tile([P, 2, 2 * P], bf16, tag="pair")
                for k, j in enumerate((jj, jj + 1)):
                    js = slice(j * P, (j + 1) * P)
                    nc.tensor.matmul(
                        out=pair[:, 0, k * P : (k + 1) * P],
                        lhsT=dp[:, js], rhs=iden_bf[:],
                        is_transpose=True, start=(k == 0), stop=False,
                    )
                    nc.tensor.matmul(
                        out=pair[:, 1, k * P : (k + 1) * P],
                        lhsT=dn[:, js], rhs=iden_bf[:],
                        is_transpose=True, start=False, stop=(k == 1),
                    )
                ps2 = slice(jj * P, (jj + 2) * P)
                nc.vector.tensor_copy(dT[:, :, ps2], pair[:])
                nc.vector.tensor_tensor(
                    out=dTm[:, :, ps2], in0=dT[:, :, ps2], in1=mf3[:, :, ps2],
                    op=mybir.AluOpType.mult,
                )

                for j in (jj, jj + 1):
                    js = slice(j * P, (j + 1) * P)
                    # vT = W_A^T dT + W_D^T dTm + [r_A; r_D] x [ones; mrow]
                    vt = ps_vt.tile([P, 2 * P], f32, tag="vt")
                    nc.tensor.matmul(
                        out=vt[:], lhsT=wab_sb[:, js], rhs=dT[:, :, js],
                        start=True, stop=False,
                    )
                    nc.tensor.matmul(
                        out=vt[:],
                        lhsT=wab_sb[:, (NT + j) * P : (NT + j + 1) * P],
                        rhs=dTm[:, :, js],
                        start=False, stop=False,
                    )
                    nc.tensor.matmul(
                        out=vt[:],
                        lhsT=rsm_sb[0:2, _R_RR + j * P : _R_RR + (j + 1) * P],
                        rhs=rsm_sb[
                            0:2, _R_RRHS + j * 2 * P : _R_RRHS + (j + 1) * 2 * P
                        ],
                        start=False, stop=True,
                    )
                    # sq[:, :, js] = vt^2   (bf16)
                    nc.scalar.activation(
                        out=sq[:, :, js], in_=vt[:],
                        func=mybir.ActivationFunctionType.Square,
                    )

                # score-diff transposes for (jj, jj+1), one PSUM group:
                #   scp[:, k, :] = sq_neg_j^T - sq_pos_j^T
                scp = ps_sc.tile([P, 2, P], f32, tag="scp")
                for k, j in enumerate((jj, jj + 1)):
                    js = slice(j * P, (j + 1) * P)
                    nc.tensor.matmul(
                        out=scp[:, k, :], lhsT=sq[:, 1, js], rhs=iden[:],
                        is_transpose=True,
                        start=(k == 0), stop=False,
                    )
                    nc.tensor.matmul(
                        out=scp[:, k, :], lhsT=sq[:, 0, js], rhs=negiden[:],
                        start=False, stop=(k == 1),
                    )
                nc.vector.tensor_reduce(
                    out=dsc[:, jj : jj + 2], in_=scp[:],
                    axis=mybir.AxisListType.X, op=mybir.AluOpType.add,
                )

                # interleave g1 reg quarter-chunks behind the tile work
                if gi == 1:
                    q = (jj - t0) // 2            # 0..2
                    qcols = (t1 - t0) * P // 3    # 256
                    sl = slice(t0 * P + q * qcols, t0 * P + (q + 1) * qcols)
                    if q == 0:
                        reg_act(sl, 9, 3 * qcols)
                    elif q == 1:
                        reg_dve(sl, 10, 3 * qcols)
                    else:
                        reg_pool(sl, 11, 3 * qcols)

        # ---- softplus tail: acc[:, 0:8] = softplus(0.5*dsc) ----
        t_abs = const.tile([P, NT], f32)
        nc.scalar.activation(
            out=t_abs[:], in_=dsc[:], func=mybir.ActivationFunctionType.Abs,
            scale=0.5,
        )
        t_exp = const.tile([P, NT], f32)
        nc.scalar.activation(
            out=t_exp[:], in_=t_abs[:], func=mybir.ActivationFunctionType.Exp,
            scale=-1.0,
        )
        t_ln = const.tile([P, NT], f32)
        nc.scalar.activation(
            out=t_ln[:], in_=t_exp[:], func=mybir.ActivationFunctionType.Ln,
            bias=1.0,
        )
        t_relu = const.tile([P, NT], f32)
        nc.vector.scalar_tensor_tensor(
            out=t_relu[:], in0=dsc[:], scalar=0.5, in1=zeros8[:],
            op0=mybir.AluOpType.mult, op1=mybir.AluOpType.max,
        )
        nc.vector.tensor_tensor(
            out=acc[:, 0:NT], in0=t_ln[:], in1=t_relu[:], op=mybir.AluOpType.add
        )

        # ---- reg scale + relation reg ----
        nc.vector.tensor_scalar_mul(
            out=acc[:, 8:12], in0=acc[:, 8:12], scalar1=0.5 * LAM
        )
        rsq_scr = const.tile([64, E], f32)
        rsq_col = const.tile([64, 1], f32)
        nc.vector.tensor_tensor_reduce(
            out=rsq_scr[:], in0=rfull_sb[0:64, :], in1=rfull_sb[0:64, :],
            scale=1.0, scalar=0.0,
            op0=mybir.AluOpType.mult, op1=mybir.AluOpType.add,
            accum_out=rsq_col[:],
        )
        nc.vector.tensor_tensor(
            out=acc[0:64, 12:13], in0=rsq_col[:], in1=cnt_sb[0:64, :],
            op=mybir.AluOpType.mult,
        )

        # ---- final scalar ----
        t_col = const.tile([P, 1], f32)
        nc.vector.tensor_reduce(
            out=t_col[:], in_=acc[:], axis=mybir.AxisListType.X,
            op=mybir.AluOpType.add,
        )
        fin_ps = ps_fin.tile([1, 1], f32)
        nc.tensor.matmul(
            out=fin_ps[:], lhsT=t_col[:], rhs=ones_col[:], start=True, stop=True
        )
        fin_sb = const.tile([1, 1], f32)
        nc.scalar.copy(fin_sb[:], fin_ps[:])
        nc.sync.dma_start(out=out[:], in_=fin_sb[:])
        if debug:
            nc.sync.dma_start(out=dbg_dsc[:], in_=dsc[:])
            nc.sync.dma_start(out=dbg_acc[:], in_=acc[:])

    nc.compile()
    return nc


def _choose_order(counts):
    """Permute relations so no 128-row window holds 2 boundaries."""
    rng = np.random.RandomState(0)
    for attempt in range(200):
        remaining = set(range(N_REL))
        orderp = []
        p = 0
        ok = True
        while remaining:
            cands = []
            for k in remaining:
                c = int(counts[k])
                viol = (
                    p > 0
                    and p + c < M
                    and (p // P) == ((p + c) // P)
                )
                if not viol:
                    cands.append((((p + c) % P), k))
            if not cands:
                ok = False
                break
            if attempt == 0:
                cands.sort()
                k = cands[-1][1]
            else:
                k = cands[rng.randint(len(cands))][1]
            orderp.append(k)
            p += int(counts[k])
            remaining.discard(k)
        if ok:
            return orderp
    raise RuntimeError("could not find a 1-boundary-per-tile relation order")


def _plan(h, r, pos_t, neg_t, relation_weight, relation_embed):
    counts = np.bincount(r, minlength=N_REL)
    perm = _choose_order(counts)
    order = np.concatenate(
        [np.flatnonzero(r == k) for k in perm if counts[k] > 0]
    ).astype(np.int64)
    assert order.shape[0] == M
    h_s = h[order]
    p_s = pos_t[order]
    n_s = neg_t[order]
    r_s = r[order]

    rw = relation_weight.astype(np.float32)
    re = relation_embed.astype(np.float32)

    import ml_dtypes

    maps = []
    for c in range(N_CORES):
        rows = slice(c * RPC, (c + 1) * RPC)
        hh = h_s[rows].reshape(NT, P).T.astype(np.int32)   # [128, NT]
        pp = p_s[rows].reshape(NT, P).T.astype(np.int32)
        nn = n_s[rows].reshape(NT, P).T.astype(np.int32)
        rc = r_s[rows]

        # grouped idx layout: per gather group [H tiles.., P tiles.., N tiles..]
        idx = np.zeros((P, 3 * NT), np.int32)
        for (t0, t1) in GROUPS:
            k = t1 - t0
            base = 3 * t0
            idx[:, base : base + k] = hh[:, t0:t1]
            idx[:, base + k : base + 2 * k] = pp[:, t0:t1]
            idx[:, base + 2 * k : base + 3 * k] = nn[:, t0:t1]

        wab = np.zeros((P, 2 * NT * P), np.float32)
        rr = np.zeros((2, NT * P), np.float32)
        rrhs = np.zeros((2, NT * 2 * P), np.float32)
        mful = np.zeros((2, NT * P), np.float32)  # [role, global col] mask
        for j in range(NT):
            tr = rc[j * P : (j + 1) * P]
            rel_a = int(tr[0])
            chg = np.flatnonzero(tr[1:] != tr[:-1])
            assert len(chg) <= 1, "tile with >1 relation boundary"
            wab[:, j * P : (j + 1) * P] = rw[rel_a]
            rr[0, j * P : (j + 1) * P] = re[rel_a]
            rrhs[0, j * 2 * P : (j + 1) * 2 * P] = 1.0
            if len(chg) == 1:
                b = int(chg[0]) + 1
                rel_b = int(tr[b])
                wab[:, (NT + j) * P : (NT + j + 1) * P] = rw[rel_b] - rw[rel_a]
                rr[1, j * P : (j + 1) * P] = re[rel_b] - re[rel_a]
                mrow = np.zeros(P, np.float32)
                mrow[b:] = 1.0
                rrhs[1, j * 2 * P : j * 2 * P + P] = mrow
                rrhs[1, j * 2 * P + P : (j + 1) * 2 * P] = mrow
                mful[:, j * P : (j + 1) * P] = mrow[None, :]

        cnt = np.zeros((P, 1), np.float32)
        core_counts = np.bincount(rc, minlength=N_REL)
        cnt[:N_REL, 0] = core_counts * (0.5 * LAM)
        rfull = np.zeros((P, E), np.float32)
        rfull[:N_REL] = re

        blob = np.zeros((P, _B_COLS), np.int32)
        blob[:, _B_IDX : _B_IDX + 3 * NT] = idx
        blob[:, _B_CNT : _B_CNT + 1] = cnt.view(np.int32)
        blob[:, _B_RFULL : _B_RFULL + E] = rfull.view(np.int32)

        rsm = np.zeros((2, 3 * NT * P), np.float32)
        rsm[:, _R_RR : _R_RR + NT * P] = rr
        rsm[:, _R_RRHS : _R_RRHS + 2 * NT * P] = rrhs

        maps.append(
            {
                "blob": blob,
                "wab": wab.astype(ml_dtypes.bfloat16),
                "rsm": rsm.astype(ml_dtypes.bfloat16),
                "mful": np.broadcast_to(
                    mful.reshape(1, 2 * NT * P), (P, 2 * NT * P)
                ).astype(ml_dtypes.bfloat16),
            }
        )
    return maps


def kernel(h, r, pos_t, neg_t, entity_embed, relation_embed, relation_weight):
    import ml_dtypes

    h = np.asarray(h).astype(np.int64)
    r = np.asarray(r).astype(np.int64)
    pos_t = np.asarray(pos_t).astype(np.int64)
    neg_t = np.asarray(neg_t).astype(np.int64)
    ent = np.ascontiguousarray(
        np.asarray(entity_embed, dtype=np.float32).astype(ml_dtypes.bfloat16)
    )
    re = np.ascontiguousarray(np.asarray(relation_embed, dtype=np.float32))
    rw = np.ascontiguousarray(np.asarray(relation_weight, dtype=np.float32))

    maps = _plan(h, r, pos_t, neg_t, rw, re)
    if "nc" not in _cache:
        _cache["nc"] = _build()
    nc = _cache["nc"]

    in_maps = [{"ent": ent, **maps[c]} for c in range(N_CORES)]

    if os.environ.get("KGE_SIM"):
        from concourse.bass_interp import CoreSim

        total = 0.0
        for c in range(N_CORES):
            sim = CoreSim(nc, trace=False)
            for name, arr in in_maps[c].items():
                sim.tensor(name)[:] = arr
            sim.simulate()
            total += float(sim.tensor("out")[0, 0])
        return np.float32(total / M)

    from concourse.bass_utils import run_bass_kernel_spmd

    res = run_bass_kernel_spmd(nc, in_maps, core_ids=list(range(N_CORES)))
    total = sum(float(res.results[c]["out"][0, 0]) for c in range(N_CORES))
    return np.float32(total / M)


# revision 22
# speedup vs baseline: 1.1263x; 1.1018x over previous
"""KGE (TransR-style) loss kernel for Trainium2, 8 NeuronCores.

Strategy (v3):
  - Host: choose a relation ORDER such that after sorting rows by relation,
    no 128-row tile contains more than one relation boundary (greedy over
    prefix sums). Rows split 1024/core -> 8 tiles/core, ZERO padding.
    Entity table converted to bf16 host-side.
  - Mixed-relation tiles are handled with data (not structure): per tile j
    the program computes  vT = W_A^T dT + W_D^T (dT*mask) + [r_A; r_D]-add
    where W_D = W_B - W_A and mask is the suffix indicator of the boundary
    row (a full [128, 2, 1024] bf16 mask tensor, loaded early).
  - Device (per core):
      * 2 batched indirect DMAs gather h/pos/neg rows (2-tile group then
        6-tile group so compute starts early).  Replaces the original 36
        gathers whose ~1us/SWDGE fixed cost dominated the baseline.
      * DVE/Pool: D_p = H-P, D_n = H-N (bf16, 2x mode)
      * PE: transpose D tiles (4 transposes / 2 tiles in one PSUM group)
      * DVE: PSUM->SBUF pair copies + masked copies (dT*mask), 2-tile wide
      * PE: per tile, 3-matmul PSUM group -> [v_pos | v_neg] (f32 PSUM)
      * ACT: Square -> sq (bf16)
      * PE: score diffs via  transpose(sq_neg) - sq_pos^T  (transpose +
        negated-identity matmul in one PSUM group)
      * DVE: reduce -> per-row score diffs [128, 8]
      * ONE Softplus activation for the whole tail (all activation funcs
        used live in the softplus_and_others table -> no mid-kernel
        activation-table loads; a dummy softplus pins the table early)
      * reg terms via ttr/Square accumulators interleaved off-path
      * final: reduce + ones-matmul -> one f32 per core; host sums / M.
  - PE warmup transposes run during the gather latency (p-state ramp).
"""

import os
from contextlib import ExitStack

import numpy as np

import concourse.bass as bass
import concourse.tile as tile
from concourse import bacc, mybir
from concourse.masks import make_identity

# Constrain the activation-table chooser: our funcs (Copy/Square/Abs/Exp/Ln/
# Identity) all exist in natural_log_exp_and_others; removing them from the
# other sets (positions preserved, so act_func_set ids stay valid) makes the
# insert_act_table_loads pass emit exactly one table load, up front, instead
# of one in the critical tail.
_ACT_PATCHED = False


def _patch_act_tables():
    global _ACT_PATCHED
    if _ACT_PATCHED:
        return
    _ACT_PATCHED = True
    orig = bacc.get_activation_tables
    target = "natural_log_exp_and_others"
    ours = {
        mybir.ActivationFunctionType.Copy,
        mybir.ActivationFunctionType.Identity,
        mybir.ActivationFunctionType.Square,
        mybir.ActivationFunctionType.Abs,
        mybir.ActivationFunctionType.Exp,
        mybir.ActivationFunctionType.Ln,
        mybir.ActivationFunctionType.Relu,
    }

    def patched(arch):
        tabs = orig(arch)
        return {
            name: (funcs if name == target else funcs - ours)
            for name, funcs in tabs.items()
        }

    bacc.get_activation_tables = patched

M = 8192
E = 128
N_ENT = 500000
N_REL = 64
LAM = 1e-5
P = 128
N_CORES = 8
RPC = M // N_CORES          # rows per core = 1024
NT = RPC // P               # tiles per core = 8
GROUPS = [(0, 2), (2, 4), (4, 6), (6, 8)]   # gather groups = pipeline blocks
N_WARM = 50                 # PE warmup transposes

f32 = mybir.dt.float32
bf16 = mybir.dt.bfloat16
i32 = mybir.dt.int32

_cache = {}

# ---- i32 blob column layout ----
_B_IDX = 0                    # [128, 3*NT] i32 gather indices (grouped)
_B_CNT = _B_IDX + 3 * NT      # [128, 1] f32 cnt (rows 0:64 used)
_B_RFULL = _B_CNT + 1         # [128, E] f32 relation_embed (rows 0:64)
_B_COLS = _B_RFULL + E

# ---- bf16 rsm blob layout: [2, 3*NT*128] ----
_R_RR = 0                     # row0 = r_A, row1 = r_D per tile
_R_RRHS = NT * P              # row0 = ones, row1 = [mrow|mrow] per tile


def _build():
    _patch_act_tables()
    nc = bacc.Bacc(
        "TRN2",
        target_bir_lowering=False,
        debug=False,
        num_devices=N_CORES,
    )

    ent = nc.dram_tensor("ent", (N_ENT, E), bf16, kind="ExternalInput").ap()
    blob = nc.dram_tensor("blob", (P, _B_COLS), i32, kind="ExternalInput").ap()
    wab = nc.dram_tensor("wab", (P, 2 * NT * P), bf16, kind="ExternalInput").ap()
    rsm = nc.dram_tensor("rsm", (2, 3 * NT * P), bf16, kind="ExternalInput").ap()
    mful = nc.dram_tensor("mful", (P, 2 * NT * P), bf16, kind="ExternalInput").ap()
    out = nc.dram_tensor("out", (1, 1), f32, kind="ExternalOutput").ap()
    debug = bool(os.environ.get("KGE_DEBUG"))
    if debug:
        dbg_dsc = nc.dram_tensor("dbg_dsc", (P, NT), f32, kind="ExternalOutput").ap()
        dbg_acc = nc.dram_tensor("dbg_acc", (P, 16), f32, kind="ExternalOutput").ap()

    with tile.TileContext(nc) as tc, ExitStack() as ctx:
        const = ctx.enter_context(tc.tile_pool(name="const", bufs=1))
        ps_pair = ctx.enter_context(tc.tile_pool(name="ps_pair", bufs=2, space="PSUM"))
        ps_vt = ctx.enter_context(tc.tile_pool(name="ps_vt", bufs=3, space="PSUM"))
        ps_sc = ctx.enter_context(tc.tile_pool(name="ps_sc", bufs=2, space="PSUM"))
        ps_fin = ctx.enter_context(tc.tile_pool(name="ps_fin", bufs=1, space="PSUM"))

        # ---- small inputs (blob first: the gathers wait on it) ----
        blob_sb = const.tile([P, _B_COLS], i32)
        nc.sync.dma_start(out=blob_sb[:], in_=blob[:])
        wab_sb = const.tile([P, 2 * NT * P], bf16)
        nc.sync.dma_start(out=wab_sb[:], in_=wab[:])
        rsm_sb = const.tile([2, 3 * NT * P], bf16)
        nc.sync.dma_start(out=rsm_sb[:], in_=rsm[:])
        mf_sb = const.tile([P, 2 * NT * P], bf16)
        # two half-transfers so neither blocks the first gather on the DMA
        # engines for long
        nc.sync.dma_start(out=mf_sb[:, : NT * P], in_=mful[:, : NT * P])
        nc.sync.dma_start(out=mf_sb[:, NT * P :], in_=mful[:, NT * P :])
        mf3 = mf_sb[:].rearrange("p (t c) -> p t c", t=2)

        idx_sb = blob_sb[:, _B_IDX : _B_IDX + 3 * NT]
        cnt_sb = blob_sb[:, _B_CNT : _B_CNT + 1].bitcast(f32)
        rfull_sb = blob_sb[:, _B_RFULL : _B_RFULL + E].bitcast(f32)

        # ---- constants ----
        ones_col = const.tile([P, 1], f32)
        nc.gpsimd.memset(ones_col[:], 1.0)

        zeros8 = const.tile([P, NT], f32)
        nc.gpsimd.memset(zeros8[:], 0.0)

        iden = const.tile([P, P], f32)
        make_identity(nc, iden[:])
        iden_bf = const.tile([P, P], bf16)
        nc.scalar.copy(iden_bf[:], iden[:])
        negiden = const.tile([P, P], f32)
        nc.vector.tensor_scalar_mul(out=negiden[:], in0=iden[:], scalar1=-1.0)
        acc = const.tile([P, 16], f32)
        nc.vector.memset(acc[:], 0.0)

        # ---- PE warmup (p-state ramp) ----
        wu_src = const.tile([P, P], bf16)
        nc.gpsimd.memset(wu_src[:], 0.0)
        for _ in range(N_WARM):
            wu_ps = ps_pair.tile([P, P], bf16, tag="pair")
            nc.tensor.matmul(
                out=wu_ps[:], lhsT=wu_src[:], rhs=iden_bf[:],
                is_transpose=True, start=True, stop=True,
            )

        # ---- gathers: X = [H | P | N] (each [128, RPC] bf16) ----
        X = const.tile([P, 3 * RPC], bf16)
        X3 = X[:].rearrange("p (r c) -> p r c", r=3)
        for (t0, t1) in GROUPS:
            nc.gpsimd.indirect_dma_start(
                out=X3[:, :, t0 * P : t1 * P],
                out_offset=None,
                in_=ent[:],
                in_offset=bass.IndirectOffsetOnAxis(
                    ap=idx_sb[:, 3 * t0 : 3 * t1], axis=0
                ),
            )

        xh = X[:, 0:RPC]
        xp = X[:, RPC : 2 * RPC]
        xn = X[:, 2 * RPC : 3 * RPC]

        # ---- working tiles ----
        dp = const.tile([P, RPC], bf16)
        dn = const.tile([P, RPC], bf16)
        dT = const.tile([P, 2, RPC], bf16)    # [pos | neg] transposed
        dTm = const.tile([P, 2, RPC], bf16)   # masked
        sq = const.tile([P, 2, RPC], f32)
        dsc = const.tile([P, NT], f32)
        xsq_a = const.tile([P, 3 * RPC // 4], f32)   # ACT square scratch
        xsq_b = const.tile([P, 3 * RPC // 4], f32)   # DVE ttr scratch

        xsq_c = const.tile([P, 3 * RPC // 4], f32)   # Pool stt scratch

        def reg_pool(sl, col, scratch_cols):
            gslab = X3[:, :, sl]
            nc.gpsimd.scalar_tensor_tensor(
                out=xsq_c[:, 0:scratch_cols], in0=gslab, scalar=1.0, in1=gslab,
                op0=mybir.AluOpType.mult, op1=mybir.AluOpType.mult,
                accum_out=acc[:, col : col + 1],
            )

        def reg_dve(sl, col, scratch_cols):
            gslab = X3[:, :, sl]
            nc.vector.tensor_tensor_reduce(
                out=xsq_b[:, 0:scratch_cols], in0=gslab, in1=gslab,
                scale=1.0, scalar=0.0,
                op0=mybir.AluOpType.mult, op1=mybir.AluOpType.add,
                accum_out=acc[:, col : col + 1],
            )

        def reg_act(sl, col, scratch_cols):
            nc.scalar.activation(
                out=xsq_a[:, 0:scratch_cols], in_=X3[:, :, sl],
                func=mybir.ActivationFunctionType.Square,
                accum_out=acc[:, col : col + 1],
            )

        def emit_sub(k):
            cs = slice(2 * k * P, (2 * k + 2) * P)
            nc.vector.tensor_tensor(
                out=dp[:, cs], in0=xh[:, cs], in1=xp[:, cs],
                op=mybir.AluOpType.subtract,
            )
            nc.vector.tensor_tensor(
                out=dn[:, cs], in0=xh[:, cs], in1=xn[:, cs],
                op=mybir.AluOpType.subtract,
            )

        pairs = [None] * 4
        vts = [None] * (2 * NT)
        scps = [None] * 4

        def emit_transposes(k):
            pair = ps_pair.tile([P, 2, 2 * P], bf16, tag="pair")
            pairs[k] = pair
            for kk, j in enumerate((2 * k, 2 * k + 1)):
                js = slice(j * P, (j + 1) * P)
                nc.tensor.matmul(
                    out=pair[:, 0, kk * P : (kk + 1) * P],
                    lhsT=dp[:, js], rhs=iden_bf[:],
                    is_transpose=True, start=(kk == 0), stop=False,
                )
                nc.tensor.matmul(
                    out=pair[:, 1, kk * P : (kk + 1) * P],
                    lhsT=dn[:, js], rhs=iden_bf[:],
                    is_transpose=True, start=False, stop=(kk == 1),
                )

        def emit_copy_mask(k):
            pair = pairs[k]
            ps2 = slice(2 * k * P, (2 * k + 2) * P)
            nc.vector.tensor_copy(dT[:, :, ps2], pair[:])
            nc.vector.tensor_tensor(
                out=dTm[:, :, ps2], in0=pair[:], in1=mf3[:, :, ps2],
                op=mybir.AluOpType.mult,
            )

        def emit_mms(k):
            for j in (2 * k, 2 * k + 1):
                js = slice(j * P, (j + 1) * P)
                vt = ps_vt.tile([P, 2 * P], f32, tag="vt")
                vts[j] = vt
                # r-add first: its inputs are ready immediately
                nc.tensor.matmul(
                    out=vt[:],
                    lhsT=rsm_sb[0:2, _R_RR + j * P : _R_RR + (j + 1) * P],
                    rhs=rsm_sb[
                        0:2, _R_RRHS + j * 2 * P : _R_RRHS + (j + 1) * 2 * P
                    ],
                    start=True, stop=False,
                )
                nc.tensor.matmul(
                    out=vt[:],
                    lhsT=wab_sb[:, (NT + j) * P : (NT + j + 1) * P],
                    rhs=dTm[:, :, js],
                    start=False, stop=False,
                )
                nc.tensor.matmul(
                    out=vt[:], lhsT=wab_sb[:, js], rhs=dT[:, :, js],
                    start=False, stop=True,
                )

        def emit_sq(k):
            for j in (2 * k, 2 * k + 1):
                js = slice(j * P, (j + 1) * P)
                nc.scalar.activation(
                    out=sq[:, :, js], in_=vts[j][:],
                    func=mybir.ActivationFunctionType.Square,
                )

        def emit_score(k):
            scp = ps_sc.tile([P, 2, P], f32, tag="scp")
            scps[k] = scp
            for kk, j in enumerate((2 * k, 2 * k + 1)):
                js = slice(j * P, (j + 1) * P)
                nc.tensor.matmul(
                    out=scp[:, kk, :], lhsT=sq[:, 1, js], rhs=iden[:],
                    is_transpose=True,
                    start=(kk == 0), stop=False,
                )
                nc.tensor.matmul(
                    out=scp[:, kk, :], lhsT=sq[:, 0, js], rhs=negiden[:],
                    start=False, stop=(kk == 1),
                )

        def emit_scred(k):
            nc.vector.tensor_reduce(
                out=dsc[:, 2 * k : 2 * k + 2], in_=scps[k][:],
                axis=mybir.AxisListType.X, op=mybir.AluOpType.add,
            )

        def emit_reg(k):
            sl = slice(2 * k * P, (2 * k + 2) * P)
            if k == 0:
                reg_dve(sl, 8, 3 * 2 * P)
            elif k == 2:
                reg_pool(sl, 10, 3 * 2 * P)
            elif k == 1:
                reg_act(sl, 9, 3 * 2 * P)
            else:
                reg_act(sl, 11, 3 * 2 * P)

        # relation reg early: it only needs the blob
        rsq_scr = const.tile([64, E], f32)
        rsq_col = const.tile([64, 1], f32)
        nc.vector.tensor_tensor_reduce(
            out=rsq_scr[:], in0=rfull_sb[0:64, :], in1=rfull_sb[0:64, :],
            scale=1.0, scalar=0.0,
            op0=mybir.AluOpType.mult, op1=mybir.AluOpType.add,
            accum_out=rsq_col[:],
        )
        nc.vector.tensor_tensor(
            out=acc[0:64, 12:13], in0=rsq_col[:], in1=cnt_sb[0:64, :],
            op=mybir.AluOpType.mult,
        )

        # software-pipelined emission: stage skew avoids head-of-line stalls
        for k in range(4):
            emit_sub(k)
            if k >= 2:
                emit_score(k - 2)
                emit_scred(k - 2)
            emit_transposes(k)
            emit_copy_mask(k)
            if k >= 1:
                emit_mms(k - 1)
                emit_sq(k - 1)
            emit_reg(k)
        emit_mms(3)
        emit_sq(3)
        emit_score(2)
        emit_scred(2)
        emit_score(3)
        emit_scred(3)

        # ---- softplus tail: acc[:, 0:8] = softplus(0.5*dsc) ----
        t_relu = const.tile([P, NT], f32)
        nc.vector.scalar_tensor_tensor(
            out=t_relu[:], in0=dsc[:], scalar=0.5, in1=zeros8[:],
            op0=mybir.AluOpType.mult, op1=mybir.AluOpType.max,
        )
        t_abs = const.tile([P, NT], f32)
        nc.vector.scalar_tensor_tensor(
            out=t_abs[:], in0=dsc[:], scalar=-0.5, in1=t_relu[:],
            op0=mybir.AluOpType.mult, op1=mybir.AluOpType.max,
        )
        t_exp = const.tile([P, NT], f32)
        nc.scalar.activation(
            out=t_exp[:], in_=t_abs[:], func=mybir.ActivationFunctionType.Exp,
            scale=-1.0,
        )
        t_ln = const.tile([P, NT], f32)
        nc.scalar.activation(
            out=t_ln[:], in_=t_exp[:], func=mybir.ActivationFunctionType.Ln,
            bias=1.0,
        )
        nc.vector.tensor_tensor(
            out=acc[:, 0:NT], in0=t_ln[:], in1=t_relu[:], op=mybir.AluOpType.add
        )

        # ---- reg scale + relation reg ----
        nc.vector.tensor_scalar_mul(
            out=acc[:, 8:12], in0=acc[:, 8:12], scalar1=0.5 * LAM
        )
        # ---- final scalar ----
        t_col = const.tile([P, 1], f32)
        nc.vector.tensor_reduce(
            out=t_col[:], in_=acc[:], axis=mybir.AxisListType.X,
            op=mybir.AluOpType.add,
        )
        fin_ps = ps_fin.tile([1, 1], f32)
        nc.tensor.matmul(
            out=fin_ps[:], lhsT=t_col[:], rhs=ones_col[:], start=True, stop=True
        )
        fin_sb = const.tile([1, 1], f32)
        nc.scalar.copy(fin_sb[:], fin_ps[:])
        nc.sync.dma_start(out=out[:], in_=fin_sb[:])
        if debug:
            nc.sync.dma_start(out=dbg_dsc[:], in_=dsc[:])
            nc.sync.dma_start(out=dbg_acc[:], in_=acc[:])

    nc.compile()
    return nc


def _choose_order(counts):
    """Permute relations so no 128-row window holds 2 boundaries."""
    rng = np.random.RandomState(0)
    for attempt in range(200):
        remaining = set(range(N_REL))
        orderp = []
        p = 0
        ok = True
        while remaining:
            cands = []
            for k in remaining:
                c = int(counts[k])
                viol = (
                    p > 0
                    and p + c < M
                    and (p // P) == ((p + c) // P)
                )
                if not viol:
                    cands.append((((p + c) % P), k))
            if not cands:
                ok = False
                break
            if attempt == 0:
                cands.sort()
                k = cands[-1][1]
            else:
                k = cands[rng.randint(len(cands))][1]
            orderp.append(k)
            p += int(counts[k])
            remaining.discard(k)
        if ok:
            return orderp
    raise RuntimeError("could not find a 1-boundary-per-tile relation order")


def _plan(h, r, pos_t, neg_t, relation_weight, relation_embed):
    counts = np.bincount(r, minlength=N_REL)
    perm = _choose_order(counts)
    order = np.concatenate(
        [np.flatnonzero(r == k) for k in perm if counts[k] > 0]
    ).astype(np.int64)
    assert order.shape[0] == M
    h_s = h[order]
    p_s = pos_t[order]
    n_s = neg_t[order]
    r_s = r[order]

    rw = relation_weight.astype(np.float32)
    re = relation_embed.astype(np.float32)

    import ml_dtypes

    maps = []
    for c in range(N_CORES):
        rows = slice(c * RPC, (c + 1) * RPC)
        hh = h_s[rows].reshape(NT, P).T.astype(np.int32)   # [128, NT]
        pp = p_s[rows].reshape(NT, P).T.astype(np.int32)
        nn = n_s[rows].reshape(NT, P).T.astype(np.int32)
        rc = r_s[rows]

        # grouped idx layout: per gather group [H tiles.., P tiles.., N tiles..]
        idx = np.zeros((P, 3 * NT), np.int32)
        for (t0, t1) in GROUPS:
            k = t1 - t0
            base = 3 * t0
            idx[:, base : base + k] = hh[:, t0:t1]
            idx[:, base + k : base + 2 * k] = pp[:, t0:t1]
            idx[:, base + 2 * k : base + 3 * k] = nn[:, t0:t1]

        wab = np.zeros((P, 2 * NT * P), np.float32)
        rr = np.zeros((2, NT * P), np.float32)
        rrhs = np.zeros((2, NT * 2 * P), np.float32)
        mful = np.zeros((2, NT * P), np.float32)  # [role, global col] mask
        for j in range(NT):
            tr = rc[j * P : (j + 1) * P]
            rel_a = int(tr[0])
            chg = np.flatnonzero(tr[1:] != tr[:-1])
            assert len(chg) <= 1, "tile with >1 relation boundary"
            wab[:, j * P : (j + 1) * P] = rw[rel_a]
            rr[0, j * P : (j + 1) * P] = re[rel_a]
            rrhs[0, j * 2 * P : (j + 1) * 2 * P] = 1.0
            if len(chg) == 1:
                b = int(chg[0]) + 1
                rel_b = int(tr[b])
                wab[:, (NT + j) * P : (NT + j + 1) * P] = rw[rel_b] - rw[rel_a]
                rr[1, j * P : (j + 1) * P] = re[rel_b] - re[rel_a]
                mrow = np.zeros(P, np.float32)
                mrow[b:] = 1.0
                rrhs[1, j * 2 * P : j * 2 * P + P] = mrow
                rrhs[1, j * 2 * P + P : (j + 1) * 2 * P] = mrow
                mful[:, j * P : (j + 1) * P] = mrow[None, :]

        cnt = np.zeros((P, 1), np.float32)
        core_counts = np.bincount(rc, minlength=N_REL)
        cnt[:N_REL, 0] = core_counts * (0.5 * LAM)
        rfull = np.zeros((P, E), np.float32)
        rfull[:N_REL] = re

        blob = np.zeros((P, _B_COLS), np.int32)
        blob[:, _B_IDX : _B_IDX + 3 * NT] = idx
        blob[:, _B_CNT : _B_CNT + 1] = cnt.view(np.int32)
        blob[:, _B_RFULL : _B_RFULL + E] = rfull.view(np.int32)

        rsm = np.zeros((2, 3 * NT * P), np.float32)
        rsm[:, _R_RR : _R_RR + NT * P] = rr
        rsm[:, _R_RRHS : _R_RRHS + 2 * NT * P] = rrhs

        maps.append(
            {
                "blob": blob,
                "wab": wab.astype(ml_dtypes.bfloat16),
                "rsm": rsm.astype(ml_dtypes.bfloat16),
                "mful": np.broadcast_to(
                    mful.reshape(1, 2 * NT * P), (P, 2 * NT * P)
                ).astype(ml_dtypes.bfloat16),
            }
        )
    return maps


def kernel(h, r, pos_t, neg_t, entity_embed, relation_embed, relation_weight):
    import ml_dtypes

    h = np.asarray(h).astype(np.int64)
    r = np.asarray(r).astype(np.int64)
    pos_t = np.asarray(pos_t).astype(np.int64)
    neg_t = np.asarray(neg_t).astype(np.int64)
    ent = np.ascontiguousarray(
        np.asarray(entity_embed, dtype=np.float32).astype(ml_dtypes.bfloat16)
    )
    re = np.ascontiguousarray(np.asarray(relation_embed, dtype=np.float32))
    rw = np.ascontiguousarray(np.asarray(relation_weight, dtype=np.float32))

    maps = _plan(h, r, pos_t, neg_t, rw, re)
    if "nc" not in _cache:
        _cache["nc"] = _build()
    nc = _cache["nc"]

    in_maps = [{"ent": ent, **maps[c]} for c in range(N_CORES)]

    if os.environ.get("KGE_SIM"):
        from concourse.bass_interp import CoreSim

        total = 0.0
        for c in range(N_CORES):
            sim = CoreSim(nc, trace=False)
            for name, arr in in_maps[c].items():
                sim.tensor(name)[:] = arr
            sim.simulate()
            total += float(sim.tensor("out")[0, 0])
        return np.float32(total / M)

    from concourse.bass_utils import run_bass_kernel_spmd

    res = run_bass_kernel_spmd(nc, in_maps, core_ids=list(range(N_CORES)))
    total = sum(float(res.results[c]["out"][0, 0]) for c in range(N_CORES))
    return np.float32(total / M)
